# revision 10
# baseline (speedup 1.0000x reference)
"""Trainium2 Bass kernel for nn_AttentionMemory (sparse_attention).

Reference computation (per batch b):
    mk = Mk[b].reshape(CK, N); qk = Qk[b].reshape(CK, N)
    affinity[m, q] = softmax_m( (2*mk[:,m]@qk[:,q] - |mk[:,m]|^2) / sqrt(CK) )

Sharding: 8 cores = 4 batches x 2 query-halves. Each core computes the full
memory (softmax) axis for 2048 of one batch's queries — no collectives.

Per-core layout: queries on partitions (16 q-tiles of 128), memory positions
on the free axis; softmax runs along the free axis so every q-tile completes
independently and output DMA streams from the start.  ScalarE's exp is the
hard floor: 8.4M elements/core at 1 elem/cycle/lane (measured 0.96 GHz) =
68 us busy; everything else is arranged to keep ScalarE saturated:

- a_sq is precomputed on the HOST as a centered fp16 row asqc = -0.5*(a_sq-C)
  and broadcast across the 128 q-partitions by a K=1 matmul with a ones
  stationary (same 1 col/cycle streaming cost as the old mksq matmul, but no
  on-device mk*mk pass, no mksq SBUF tile, fewer LDWEIGHTS, and the input
  pipeline only gates on mk itself).  The center C folds into the exp bias:
  exp(SCALE*ps + BIAS), BIAS = -0.5*C*SCALE.
- Per q-tile the 4096-wide row is built in two [128, 2048] PSUM tiles
  (4 banks each, ping-pong): ps = matmul(ones1,asqc) accum matmul(qk_t, mk).
  fp16 matmuls stream 512 cols (1024-wide fp16 fails the s3d3 ISA check).
- ScalarE: one 2048-wide exp per PSUM tile. Only half A carries accum_out
  (row-sum): the ACTIVATION_READ_ACCUMULATOR aux op costs ~340 ns of
  serialized ScalarE time, so half B's row-sum runs on DVE instead
  (tensor_reduce, 1x mode, ~2.2 us — DVE has the slack; ScalarE does not).
  The logits are bounded (~[-30, +8]) so no max-subtraction pass is needed.
- DVE: sumB + add + reciprocal, then two in-place [128, 2048] bf16
  tensor_scalar multiplies normalize exp_t; each half DMAs out as soon as it
  is scaled.  The host casts bf16->fp32 and transposes while gathering
  (bf16 probabilities add ~0.4% error; budget is 2e-2).
- Last q-tile: both halves use accum_out and the B-half normalize/DMA is
  split in two so the post-ACT tail is short.
- Input DMAs issue from four different engine queues in parallel so all
  inputs land ~1.5 us after the preamble instead of serializing on SP.
- A short warmup matmul burst starts the PE HAM clock ramp (1.2 -> 2.4 GHz
  needs ~3.4 us of sustained activity) while the first mk chunks load.

Walrus caps instructions at one sync wait; _strip_self_waits spills extra
waits onto single-wait Drain instructions (semantically equivalent — waits
are an AND over monotonic semaphores, executed in order on one sequencer).
"""
import math
import numpy as np

import bass_rust
from concourse import bass, tile, mybir
from concourse.alu_op_type import AluOpType
from concourse.bass_utils import run_bass_kernel_spmd

B, CK, HH, WW = 4, 128, 64, 64
N = HH * WW            # 4096 memory positions / queries per batch
QH = N // 2            # 2048 queries per core
N_CORES = 8
QTILE = 128            # queries per q-tile (PSUM partition dim)
MCHUNK = 512           # memory cols per matmul (one PSUM bank of fp32)
MHALF = 2048           # memory cols per PSUM tile / exp instruction
ASQ_C = 128.0          # host centering constant for a_sq (E[a_sq] = CK)
SCALE = 2.0 / math.sqrt(CK)
BIAS = -0.5 * ASQ_C * SCALE
N_WARM = 44            # HAM warmup matmuls: the PE clock ramp (1.2->2.4 GHz)
                       # needs ~4us of SUSTAINED activity (44 x ~107ns), and
                       # any later PE idle gap over ~1.5us drops it back with
                       # no recovery (steady state never has 3.4us of
                       # continuous busy, so it would stay at 1.2 GHz).  The
                       # warmup must also END no earlier than the first real
                       # matmul's operands (asq + mk chunk 0, ~12.5us).
F32 = mybir.dt.float32
F16 = mybir.dt.float16
BF16 = mybir.dt.bfloat16


def _build():
    nc = bass.Bass("TRN2", target_bir_lowering=False, debug=False,
                   num_devices=N_CORES)
    mk_d = nc.dram_tensor("mk", [CK, N], F16, kind="ExternalInput").ap()
    qk_d = nc.dram_tensor("qk", [CK, QH], F16, kind="ExternalInput").ap()
    asq_d = nc.dram_tensor("asq", [1, N], F16, kind="ExternalInput").ap()
    out_d = nc.dram_tensor("out", [QH, N], BF16, kind="ExternalOutput").ap()

    n_qt = QH // QTILE          # 16
    with tile.TileContext(nc) as tc:
        with tc.tile_pool(name="inp", bufs=1) as inp_pool, \
             tc.tile_pool(name="exp", bufs=3) as exp_pool, \
             tc.tile_pool(name="small", bufs=8) as small_pool, \
             tc.tile_pool(name="psum", bufs=2, space="PSUM") as psum_pool:

            mk_sb = inp_pool.tile([CK, N], F16, tag="mk")
            qk_sb = inp_pool.tile([CK, QH], F16, tag="qk")
            asq_sb = inp_pool.tile([1, N], F16, tag="asq")
            ones1 = inp_pool.tile([1, QTILE], F16, tag="ones1")
            warm = inp_pool.tile([128, QTILE], F16, tag="warm")
            biasc = inp_pool.tile([QTILE, 1], F32, tag="biasc")
            nc.vector.memset(ones1[:], 1.0)
            nc.vector.memset(warm[:], -0.5)
            nc.vector.memset(biasc[:], BIAS)

            # Parallel input DMAs on two HW-DGE queues: mk streams on the SP
            # queue in consumption order (its chunks gate the ab matmuls
            # just-in-time); asq/qk ride the Activation queue, which is idle
            # until the first ACT ~6us later.  GpSimd's software DGE measured
            # too slow/jittery (a 1.7us-late mk chunk stalled the PE and
            # permanently dropped the HAM clock to 1.2 GHz).
            # Each queue serves ~one DMA per 1.2us (first lands ~12.5us after
            # a ~4.5us DGE latency), so interleave mk across BOTH queues to
            # match the PE's consumption order: each chunk must land before
            # its ab matmuls or the HAM clock drops (mk3 arriving 0.6us late
            # cost 35% PE clock for the whole run).
            nc.scalar.dma_start(out=asq_sb[:], in_=asq_d[:])
            nc.scalar.dma_start(out=qk_sb[:, 0:QTILE], in_=qk_d[:, 0:QTILE])
            nc.sync.dma_start(out=mk_sb[:, 0:1024], in_=mk_d[:, 0:1024])
            nc.scalar.dma_start(out=mk_sb[:, 1024:2048],
                                in_=mk_d[:, 1024:2048])
            nc.sync.dma_start(out=mk_sb[:, 2048:3072], in_=mk_d[:, 2048:3072])
            nc.scalar.dma_start(out=mk_sb[:, 3072:4096],
                                in_=mk_d[:, 3072:4096])
            nc.sync.dma_start(out=qk_sb[:, QTILE:QH], in_=qk_d[:, QTILE:QH])

            # Warm the PE's HAM clock gate with throwaway matmuls that only
            # need the memset warm tile, overlapping the input-DMA wait.
            warm_ps = psum_pool.tile([QTILE, MHALF], F32, tag="ps")
            for w in range(N_WARM):
                nc.tensor.matmul(warm_ps[:, 0:QTILE], warm[:], warm[:],
                                 start=True, stop=True)

            for t in range(n_qt):
                qk_t = qk_sb[:, t * QTILE:(t + 1) * QTILE]
                exp_t = exp_pool.tile([QTILE, N], BF16, tag="exp")
                parts = small_pool.tile([QTILE, 2], F32, tag="parts")
                s_t = small_pool.tile([QTILE, 1], F32, tag="S")
                rec_t = small_pool.tile([QTILE, 1], F32, tag="rec")
                last = t == n_qt - 1
                for h in range(2):
                    ps = psum_pool.tile([QTILE, MHALF], F32, tag="ps")
                    for c in range(4):
                        m0 = h * MHALF + c * MCHUNK
                        nc.tensor.matmul(ps[:, c * MCHUNK:(c + 1) * MCHUNK],
                                         ones1[:], asq_sb[:, m0:m0 + MCHUNK],
                                         start=True, stop=False)
                    for c in range(4):
                        m0 = h * MHALF + c * MCHUNK
                        nc.tensor.matmul(ps[:, c * MCHUNK:(c + 1) * MCHUNK],
                                         qk_t, mk_sb[:, m0:m0 + MCHUNK],
                                         start=False, stop=True)
                    accum = parts[:, h:h + 1] if (h == 0 or last) else None
                    nc.scalar.activation(
                        exp_t[:, h * MHALF:(h + 1) * MHALF], ps[:],
                        mybir.ActivationFunctionType.Exp, scale=SCALE,
                        bias=biasc[:], accum_out=accum)
                # Denominator: half A from ScalarE's accumulator, half B on
                # DVE (keeps a 340ns ACTIVATION_READ_ACCUMULATOR off the
                # saturated ScalarE).  Last tile: both halves from ScalarE so
                # the post-ACT tail skips the 2.2us DVE reduce.
                if last:
                    nc.vector.tensor_add(s_t[:], parts[:, 0:1], parts[:, 1:2])
                else:
                    sumb = small_pool.tile([QTILE, 1], F32, tag="sumb")
                    nc.vector.tensor_reduce(sumb[:], exp_t[:, MHALF:N],
                                            mybir.AxisListType.X,
                                            AluOpType.add)
                    nc.vector.tensor_add(s_t[:], parts[:, 0:1], sumb[:])
                nc.vector.reciprocal(rec_t[:], s_t[:])
                # Normalize in place + store per half: output DMA of half h
                # starts while half h+1 is still being scaled.
                chunks = ((0, MHALF), (MHALF, N)) if not last else \
                    ((0, MHALF), (MHALF, 3072), (3072, N))
                for lo, hi in chunks:
                    nc.vector.tensor_scalar_mul(
                        exp_t[:, lo:hi], exp_t[:, lo:hi], rec_t[:])
                    nc.sync.dma_start(
                        out=out_d[t * QTILE:(t + 1) * QTILE, lo:hi],
                        in_=exp_t[:, lo:hi])
    _strip_self_waits(nc)
    return nc


def _strip_self_waits(nc):
    """Walrus rejects instructions carrying more than one sync wait.

    Conservative fix: for any instruction with N>1 waits, keep the last wait
    on the instruction and spill the other N-1 onto single-wait Drain
    instructions inserted immediately before it on the same engine. All waits
    still execute, in program order, on the same sequencer; semaphores are
    monotonic so splitting an AND of waits into a sequence is equivalent.
    """
    for fn in nc.m.functions:
        for blk in fn.blocks:
            il = blk.instructions
            new_il = []
            changed = False
            for ins in il:
                si = getattr(ins, "sync_info", None)
                if si is not None and len(si.on_wait) > 1:
                    changed = True
                    waits = list(si.on_wait)
                    for k, w in enumerate(waits[:-1]):
                        d = mybir.InstDrain(
                            name=f"{ins.name}_w{k}",
                            ins=[], outs=[], bass_is_fusable=False)
                        d.engine = ins.engine
                        d.sync_info = bass_rust.SyncInfo(on_wait=[w],
                                                         on_update=[])
                        new_il.append(d)
                    ins.sync_info = bass_rust.SyncInfo(on_wait=[waits[-1]],
                                                      on_update=si.on_update)
                new_il.append(ins)
            if changed:
                blk.instructions = new_il


_NC_CACHE = None


def _make_in_maps(Mk: np.ndarray, Qk: np.ndarray) -> list[dict]:
    Mk = np.ascontiguousarray(np.asarray(Mk), dtype=np.float32)
    Qk = np.ascontiguousarray(np.asarray(Qk), dtype=np.float32)
    in_maps = []
    asq_rows = {}
    for b in range(B):
        mkb = Mk[b].reshape(CK, N)
        asq = np.sum(mkb * mkb, axis=0)                  # [N] fp32
        asq_rows[b] = np.ascontiguousarray(
            (-0.5 * (asq - ASQ_C)).astype(np.float16).reshape(1, N))
    for c in range(N_CORES):
        b, half = c // 2, c % 2
        mk = np.ascontiguousarray(Mk[b].reshape(CK, N).astype(np.float16))
        qk = np.ascontiguousarray(
            Qk[b].reshape(CK, N)[:, half * QH:(half + 1) * QH]
            .astype(np.float16))
        in_maps.append({"mk": mk, "qk": qk, "asq": asq_rows[b]})
    return in_maps


def kernel(Mk: np.ndarray, Qk: np.ndarray) -> np.ndarray:
    global _NC_CACHE
    if _NC_CACHE is None:
        _NC_CACHE = _build()
    nc = _NC_CACHE

    in_maps = _make_in_maps(Mk, Qk)

    res = run_bass_kernel_spmd(nc, in_maps, core_ids=list(range(N_CORES)))

    out = np.empty((B, N, N), dtype=np.float32)
    for c in range(N_CORES):
        b, half = c // 2, c % 2
        out[b, :, half * QH:(half + 1) * QH] = \
            res.results[c]["out"].astype(np.float32).T
    return out


# revision 13
# speedup vs baseline: 1.0023x; 1.0023x over previous
"""Trainium2 Bass kernel for nn_AttentionMemory (sparse_attention).

Reference computation (per batch b):
    mk = Mk[b].reshape(CK, N); qk = Qk[b].reshape(CK, N)
    affinity[m, q] = softmax_m( (2*mk[:,m]@qk[:,q] - |mk[:,m]|^2) / sqrt(CK) )

Sharding: 8 cores = 4 batches x 2 query-halves. Each core computes the full
memory (softmax) axis for 2048 of one batch's queries — no collectives.

Per-core layout: queries on partitions (16 q-tiles of 128), memory positions
on the free axis; softmax runs along the free axis so every q-tile completes
independently and output DMA streams from the start.  ScalarE's exp is the
hard floor: 8.4M elements/core at 1 elem/cycle/lane (measured 0.96 GHz) =
68 us busy; everything else is arranged to keep ScalarE saturated:

- a_sq is precomputed on the HOST as a centered fp16 row asqc = -0.5*(a_sq-C)
  and broadcast across the 128 q-partitions by a K=1 matmul with a ones
  stationary (same 1 col/cycle streaming cost as the old mksq matmul, but no
  on-device mk*mk pass, no mksq SBUF tile, fewer LDWEIGHTS, and the input
  pipeline only gates on mk itself).  The center C folds into the exp bias:
  exp(SCALE*ps + BIAS), BIAS = -0.5*C*SCALE.
- Per q-tile the 4096-wide row is built in two [128, 2048] PSUM tiles
  (4 banks each, ping-pong): ps = matmul(ones1,asqc) accum matmul(qk_t, mk).
  fp16 matmuls stream 512 cols (1024-wide fp16 fails the s3d3 ISA check).
- ScalarE: one 2048-wide exp per PSUM tile. Only half A carries accum_out
  (row-sum): the ACTIVATION_READ_ACCUMULATOR aux op costs ~340 ns of
  serialized ScalarE time, so half B's row-sum runs on DVE instead
  (tensor_reduce, 1x mode, ~2.2 us — DVE has the slack; ScalarE does not).
  The logits are bounded (~[-30, +8]) so no max-subtraction pass is needed.
- DVE: sumB + add + reciprocal, then two in-place [128, 2048] bf16
  tensor_scalar multiplies normalize exp_t; each half DMAs out as soon as it
  is scaled.  The host casts bf16->fp32 and transposes while gathering
  (bf16 probabilities add ~0.4% error; budget is 2e-2).
- Last q-tile: both halves use accum_out and the B-half normalize/DMA is
  split in two so the post-ACT tail is short.
- Input DMAs issue from four different engine queues in parallel so all
  inputs land ~1.5 us after the preamble instead of serializing on SP.
- A short warmup matmul burst starts the PE HAM clock ramp (1.2 -> 2.4 GHz
  needs ~3.4 us of sustained activity) while the first mk chunks load.

Walrus caps instructions at one sync wait; _strip_self_waits spills extra
waits onto single-wait Drain instructions (semantically equivalent — waits
are an AND over monotonic semaphores, executed in order on one sequencer).
"""
import math
import numpy as np

import bass_rust
from concourse import bass, tile, mybir
from concourse.alu_op_type import AluOpType
from concourse.bass_utils import run_bass_kernel_spmd

B, CK, HH, WW = 4, 128, 64, 64
N = HH * WW            # 4096 memory positions / queries per batch
QH = N // 2            # 2048 queries per core
N_CORES = 8
QTILE = 128            # queries per q-tile (PSUM partition dim)
MCHUNK = 512           # memory cols per matmul (one PSUM bank of fp32)
MHALF = 2048           # memory cols per PSUM tile / exp instruction
ASQ_C = 128.0          # host centering constant for a_sq (E[a_sq] = CK)
SCALE = 2.0 / math.sqrt(CK)
BIAS = -0.5 * ASQ_C * SCALE
N_WARM = 52            # HAM warmup matmuls: the PE clock ramp (1.2->2.4 GHz)
                       # needs ~4us of SUSTAINED activity (~107ns each), and
                       # any later PE idle gap of ~1us drops it back with no
                       # recovery (steady state never has 3.4us of continuous
                       # busy, so the whole run would stay at 1.2 GHz: +35us).
                       # The warmup must also END no earlier than the first
                       # real matmul's operands land (asq + mk0, ~13.3us).
N_PAD = (6, 6, 4)      # extra pad matmuls before (t0.B, t1.A, t1.B) asq
                       # groups: absorb input-DMA arrival jitter so the PE
                       # never idles >0.6us while mk chunks stream in.
F32 = mybir.dt.float32
F16 = mybir.dt.float16
BF16 = mybir.dt.bfloat16


def _build():
    nc = bass.Bass("TRN2", target_bir_lowering=False, debug=False,
                   num_devices=N_CORES)
    mk_d = nc.dram_tensor("mk", [CK, N], F16, kind="ExternalInput").ap()
    qk_d = nc.dram_tensor("qk", [CK, QH], F16, kind="ExternalInput").ap()
    asq_d = nc.dram_tensor("asq", [1, N], F16, kind="ExternalInput").ap()
    out_d = nc.dram_tensor("out", [QH, N], BF16, kind="ExternalOutput").ap()

    n_qt = QH // QTILE          # 16
    with tile.TileContext(nc) as tc:
        with tc.tile_pool(name="inp", bufs=1) as inp_pool, \
             tc.tile_pool(name="exp", bufs=3) as exp_pool, \
             tc.tile_pool(name="small", bufs=8) as small_pool, \
             tc.tile_pool(name="psum", bufs=2, space="PSUM") as psum_pool:

            mk_sb = inp_pool.tile([CK, N], F16, tag="mk")
            qk_sb = inp_pool.tile([CK, QH], F16, tag="qk")
            asq_sb = inp_pool.tile([1, N], F16, tag="asq")
            ones1 = inp_pool.tile([1, QTILE], F16, tag="ones1")
            warm = inp_pool.tile([128, QTILE], F16, tag="warm")
            biasc = inp_pool.tile([QTILE, 1], F32, tag="biasc")
            nc.vector.memset(ones1[:], 1.0)
            nc.vector.memset(warm[:], -0.5)
            nc.vector.memset(biasc[:], BIAS)

            # Parallel input DMAs on two HW-DGE queues: mk streams on the SP
            # queue in consumption order (its chunks gate the ab matmuls
            # just-in-time); asq/qk ride the Activation queue, which is idle
            # until the first ACT ~6us later.  GpSimd's software DGE measured
            # too slow/jittery (a 1.7us-late mk chunk stalled the PE and
            # permanently dropped the HAM clock to 1.2 GHz).
            # mk streams on the SP queue in consumption order (~1.2us per
            # 256KB chunk after a ~4.5us DGE latency; the first chunk lands
            # ~13.4us).  The tiny asq row + qk ride the Activation queue,
            # idle until its first ACT ~7us later.
            nc.scalar.dma_start(out=asq_sb[:], in_=asq_d[:])
            nc.scalar.dma_start(out=qk_sb[:, 0:QTILE], in_=qk_d[:, 0:QTILE])
            nc.sync.dma_start(out=mk_sb[:, 0:1024], in_=mk_d[:, 0:1024])
            nc.sync.dma_start(out=mk_sb[:, 1024:2048], in_=mk_d[:, 1024:2048])
            nc.sync.dma_start(out=mk_sb[:, 2048:3072], in_=mk_d[:, 2048:3072])
            nc.sync.dma_start(out=mk_sb[:, 3072:4096], in_=mk_d[:, 3072:4096])
            nc.scalar.dma_start(out=qk_sb[:, QTILE:QH], in_=qk_d[:, QTILE:QH])

            # Warm the PE's HAM clock gate with throwaway matmuls that only
            # need the memset warm tile, overlapping the input-DMA wait.
            warm_ps = psum_pool.tile([QTILE, MHALF], F32, tag="ps")
            for w in range(N_WARM):
                nc.tensor.matmul(warm_ps[:, 0:QTILE], warm[:], warm[:],
                                 start=True, stop=True)

            for t in range(n_qt):
                qk_t = qk_sb[:, t * QTILE:(t + 1) * QTILE]
                exp_t = exp_pool.tile([QTILE, N], BF16, tag="exp")
                parts = small_pool.tile([QTILE, 2], F32, tag="parts")
                s_t = small_pool.tile([QTILE, 1], F32, tag="S")
                rec_t = small_pool.tile([QTILE, 1], F32, tag="rec")
                last = t == n_qt - 1
                for h in range(2):
                    ps = psum_pool.tile([QTILE, MHALF], F32, tag="ps")
                    # Jitter-absorbing pads while input DMAs stream in: keep
                    # the PE busy so the HAM clock holds.  Safe: the asq
                    # matmuls below overwrite this region with start=True.
                    pad_idx = 2 * t + h - 1
                    if 0 <= pad_idx < len(N_PAD):
                        for _ in range(N_PAD[pad_idx]):
                            nc.tensor.matmul(ps[:, 0:QTILE], warm[:], warm[:],
                                             start=True, stop=True)
                    for c in range(4):
                        m0 = h * MHALF + c * MCHUNK
                        nc.tensor.matmul(ps[:, c * MCHUNK:(c + 1) * MCHUNK],
                                         ones1[:], asq_sb[:, m0:m0 + MCHUNK],
                                         start=True, stop=False)
                    for c in range(4):
                        m0 = h * MHALF + c * MCHUNK
                        nc.tensor.matmul(ps[:, c * MCHUNK:(c + 1) * MCHUNK],
                                         qk_t, mk_sb[:, m0:m0 + MCHUNK],
                                         start=False, stop=True)
                    accum = parts[:, h:h + 1] if (h == 0 or last) else None
                    nc.scalar.activation(
                        exp_t[:, h * MHALF:(h + 1) * MHALF], ps[:],
                        mybir.ActivationFunctionType.Exp, scale=SCALE,
                        bias=biasc[:], accum_out=accum)
                # Denominator: half A from ScalarE's accumulator, half B on
                # DVE (keeps a 340ns ACTIVATION_READ_ACCUMULATOR off the
                # saturated ScalarE).  Last tile: both halves from ScalarE so
                # the post-ACT tail skips the 2.2us DVE reduce.
                if last:
                    nc.vector.tensor_add(s_t[:], parts[:, 0:1], parts[:, 1:2])
                else:
                    sumb = small_pool.tile([QTILE, 1], F32, tag="sumb")
                    nc.vector.tensor_reduce(sumb[:], exp_t[:, MHALF:N],
                                            mybir.AxisListType.X,
                                            AluOpType.add)
                    nc.vector.tensor_add(s_t[:], parts[:, 0:1], sumb[:])
                nc.vector.reciprocal(rec_t[:], s_t[:])
                # Normalize in place + store per half: output DMA of half h
                # starts while half h+1 is still being scaled.
                chunks = ((0, MHALF), (MHALF, N)) if not last else \
                    ((0, MHALF), (MHALF, 3072), (3072, N))
                for lo, hi in chunks:
                    nc.vector.tensor_scalar_mul(
                        exp_t[:, lo:hi], exp_t[:, lo:hi], rec_t[:])
                    nc.sync.dma_start(
                        out=out_d[t * QTILE:(t + 1) * QTILE, lo:hi],
                        in_=exp_t[:, lo:hi])
    _strip_self_waits(nc)
    return nc


def _strip_self_waits(nc):
    """Walrus rejects instructions carrying more than one sync wait.

    Conservative fix: for any instruction with N>1 waits, keep the last wait
    on the instruction and spill the other N-1 onto single-wait Drain
    instructions inserted immediately before it on the same engine. All waits
    still execute, in program order, on the same sequencer; semaphores are
    monotonic so splitting an AND of waits into a sequence is equivalent.
    """
    for fn in nc.m.functions:
        for blk in fn.blocks:
            il = blk.instructions
            new_il = []
            changed = False
            for ins in il:
                si = getattr(ins, "sync_info", None)
                if si is not None and len(si.on_wait) > 1:
                    changed = True
                    waits = list(si.on_wait)
                    for k, w in enumerate(waits[:-1]):
                        d = mybir.InstDrain(
                            name=f"{ins.name}_w{k}",
                            ins=[], outs=[], bass_is_fusable=False)
                        d.engine = ins.engine
                        d.sync_info = bass_rust.SyncInfo(on_wait=[w],
                                                         on_update=[])
                        new_il.append(d)
                    ins.sync_info = bass_rust.SyncInfo(on_wait=[waits[-1]],
                                                      on_update=si.on_update)
                new_il.append(ins)
            if changed:
                blk.instructions = new_il


_NC_CACHE = None


def _make_in_maps(Mk: np.ndarray, Qk: np.ndarray) -> list[dict]:
    Mk = np.ascontiguousarray(np.asarray(Mk), dtype=np.float32)
    Qk = np.ascontiguousarray(np.asarray(Qk), dtype=np.float32)
    in_maps = []
    asq_rows = {}
    for b in range(B):
        mkb = Mk[b].reshape(CK, N)
        asq = np.sum(mkb * mkb, axis=0)                  # [N] fp32
        asq_rows[b] = np.ascontiguousarray(
            (-0.5 * (asq - ASQ_C)).astype(np.float16).reshape(1, N))
    for c in range(N_CORES):
        b, half = c // 2, c % 2
        mk = np.ascontiguousarray(Mk[b].reshape(CK, N).astype(np.float16))
        qk = np.ascontiguousarray(
            Qk[b].reshape(CK, N)[:, half * QH:(half + 1) * QH]
            .astype(np.float16))
        in_maps.append({"mk": mk, "qk": qk, "asq": asq_rows[b]})
    return in_maps


def kernel(Mk: np.ndarray, Qk: np.ndarray) -> np.ndarray:
    global _NC_CACHE
    if _NC_CACHE is None:
        _NC_CACHE = _build()
    nc = _NC_CACHE

    in_maps = _make_in_maps(Mk, Qk)

    res = run_bass_kernel_spmd(nc, in_maps, core_ids=list(range(N_CORES)))

    out = np.empty((B, N, N), dtype=np.float32)
    for c in range(N_CORES):
        b, half = c // 2, c % 2
        out[b, :, half * QH:(half + 1) * QH] = \
            res.results[c]["out"].astype(np.float32).T
    return out
